# revision 44
# baseline (speedup 1.0000x reference)
"""Multi-head attention (B=2, N=4096, C=512, H=8) on 8 trn2 NeuronCores.

Sharding: core -> (batch b = core//4, head-pair hp = core%4), i.e. data
parallel over B and tensor parallel over the 8 heads (2 heads per core),
with column-sharded qkv weights and row-sharded proj weights. Each core
returns a partial projection output [4096, 512]; the host sums the 4
head-pair partials per batch and adds proj_b.

Per-core device kernel (flash-style, nothing N^2 ever hits HBM):
  qT/kT  [128(=2 heads x 64 feat), 4096]  <- wqk^T @ x^T   (bf16 matmuls)
  qT2/kT2: head-swapped copies (h0 in partitions 64:128) so consecutive
    key m-tiles contract in disjoint 64-row PE groups -> adjacent K=64
    score matmuls run CONCURRENTLY in the array (row tiling, ~2x scores)
  v_sb   [128 keys, 32 m-tiles, 65]       <- x^T^T @ wv (+bias), ones col
  per (query-group g of 512 queries, head h), chunks of CH=3 key m-tiles:
    S^T chunk [128 keys, 3*512 q] in PSUM <- kT_m-x-qT  (scores matmuls)
    E = exp(SCALE * S^T) on the ACT engine -> SBUF bf16 (one ACTIVATE/chunk;
    the ACT engine is the throughput wall: 33.5M exps/core at 1 elem/
    cycle/lane x 1.2 GHz ~= 218us + 293ns/ACTIVATE overhead)
    out^T [65, 512] PSUM += v_aug-x-E  (row 64 = softmax denominator, free
    via the ones column; accumulation lags the exp stream by a skew that
    starts at 14 chunks -- so the qk/v projections own the PE during item
    0 -- and decays to 2)
  per (g, h) tail, deferred via a pending queue popped every OTHER chunk
  (each piece is a PE->DVE->PE round-trip through one PSUM slot; popping
  faster stalls the strict-order PE queue and HAM-cools the clock):
    drain out^T to SBUF; transpose denom row to [128, 4] with N=1 matmuls;
    128-lane reciprocal; per-head proj of the UNNORMALIZED out^T; fused
    scale-by-1/denom + head-sum via scalar_tensor_tensor on DVE; DMA out.

Scheduling notes (measured on HW): 8 warmup matmuls release the HAM
clock gate during the ~8.5us queue-boot window; all x tiles prefetch up
front; q/k/v projections are demand-paced (kproj by the key-m frontier,
vproj voluntarily at 1/chunk from chunk 2 -- item-0 chunks must stay PE-
dense or idle accumulates in the HAM window and re-throttles the clock --
then 3/chunk from chunk 11). The next group's q-projection prefetches at
ci%22==13 so its DVE copies clear before the group boundary.
PSUM budget: 6 banks score double-buffer + 2 transient/AV banks = 8.
The device power-throttles run-to-run (ACT busy 257 vs 308us bimodal);
compare only runs with similar scalar-engine busy time.
"""

import numpy as np

_state = {}

B, N, C, H, DH = 2, 4096, 512, 8, 64
SCALE = DH ** -0.5
GQ = 512          # queries per group
NG = N // GQ      # 8 groups
MT = N // 128     # 32 key m-tiles
CH = 3            # m-tiles per exp chunk


def _build_nc(debug=False):
    from contextlib import ExitStack

    import concourse.bacc as bacc
    import concourse.tile as tile
    from concourse import mybir

    bf16 = mybir.dt.bfloat16
    f32 = mybir.dt.float32
    f32r = mybir.dt.float32r
    EXP = mybir.ActivationFunctionType.Exp

    nc = bacc.Bacc(None, target_bir_lowering=False)
    with tile.TileContext(nc) as tc, ExitStack() as ctx:
        dram = ctx.enter_context(tc.tile_pool(name="dram", bufs=1, space="DRAM"))
        xt_d = dram.tile([C, N], bf16, kind="ExternalInput", name="xt",
                         uniquify=False, tag="dxt")
        wqk_d = dram.tile([C, 256], bf16, kind="ExternalInput", name="wqk",
                          uniquify=False, tag="dwqk")
        bqk_d = dram.tile([128, 2], f32, kind="ExternalInput", name="bqk",
                          uniquify=False, tag="dbqk")
        wv_d = dram.tile([C, 128], bf16, kind="ExternalInput", name="wv",
                         uniquify=False, tag="dwv")
        bv_d = dram.tile([128, 128], bf16, kind="ExternalInput", name="bv",
                         uniquify=False, tag="dbv")
        pw_d = dram.tile([64, 1024], bf16, kind="ExternalInput", name="pw2",
                         uniquify=False, tag="dpw")
        out_d = dram.tile([N, C], f32, kind="ExternalOutput", name="out",
                          uniquify=False, tag="dout")
        if debug:
            dbg_qT = dram.tile([128, N], bf16, kind="ExternalOutput",
                               name="dbg_qT", uniquify=False, tag="dbg_qT")
            dbg_kT = dram.tile([128, N], bf16, kind="ExternalOutput",
                               name="dbg_kT", uniquify=False, tag="dbg_kT")
            dbg_v = dram.tile([128, MT, 130], bf16, kind="ExternalOutput",
                              name="dbg_v", uniquify=False, tag="dbg_v")
            dbg_e = dram.tile([128, 3 * GQ], bf16, kind="ExternalOutput",
                              name="dbg_e", uniquify=False, tag="dbg_e")
            dbg_at = dram.tile([64, GQ], bf16, kind="ExternalOutput",
                               name="dbg_at", uniquify=False, tag="dbg_at")
            dbg_rb = dram.tile([1, GQ], bf16, kind="ExternalOutput",
                               name="dbg_rb", uniquify=False, tag="dbg_rb")

        const = ctx.enter_context(tc.tile_pool(name="const", bufs=1))
        wqk_sb = const.tile([128, 4, 256], bf16, name="wqk_sb", tag="wqk_sb")
        nc.gpsimd.dma_start(wqk_sb[:], wqk_d.rearrange("(k p) f -> p k f", p=128))
        wv_sb = const.tile([128, 4, 128], bf16, name="wv_sb", tag="wv_sb")
        nc.gpsimd.dma_start(wv_sb[:], wv_d.rearrange("(k p) f -> p k f", p=128))
        bqk_sb = const.tile([128, 2], f32, name="bqk_sb", tag="bqk_sb")
        nc.gpsimd.dma_start(bqk_sb[:], bqk_d[:])
        bv_sb = const.tile([128, 128], bf16, name="bv_sb", tag="bv_sb")
        nc.gpsimd.dma_start(bv_sb[:], bv_d[:])
        pw_sb = const.tile([64, 1024], bf16, name="pw_sb", tag="pw_sb")
        nc.gpsimd.dma_start(pw_sb[:], pw_d[:])
        ones_sb = const.tile([65, 128], bf16, name="ones_sb", tag="ones_sb")
        nc.vector.memset(ones_sb[:], 1.0)
        # PE warmup: ~34 dummy matmuls on a self-contained SBUF tile keep the
        # PE busy through the ~9us DMA/boot window so the HAM clock gate is
        # released (2.4 GHz) before the first real matmul issues.
        warm_sb = const.tile([128, 512], bf16, name="warm_sb", tag="warm_sb")
        nc.vector.memset(warm_sb[:], 1.0)

        persist = ctx.enter_context(tc.tile_pool(name="persist", bufs=1))
        qT = persist.tile([128, N], bf16, name="qT", tag="qT")
        kT = persist.tile([128, N], bf16, name="kT", tag="kT")
        # head-swapped copies (h0 in partitions 64:128, h1 in 0:64): lets
        # consecutive key m-tiles use disjoint 64-row PE groups so their
        # K=64 score matmuls run concurrently in the array (row tiling).
        qT2 = persist.tile([128, N], bf16, name="qT2", tag="qT2")
        kT2 = persist.tile([128, N], bf16, name="kT2", tag="kT2")
        vsb = persist.tile([128, MT, 130], bf16, name="vsb", tag="vsb")
        vones = vsb.rearrange("p m (a b) -> p m a b", a=2)
        nc.vector.memset(vones[:, :, 0, 64:65], 1.0)
        nc.vector.memset(vones[:, :, 1, 64:65], 1.0)

        xpool = ctx.enter_context(tc.tile_pool(name="xp", bufs=8))
        spool = ctx.enter_context(tc.tile_pool(name="sp", bufs=2, space="PSUM"))
        apool = ctx.enter_context(tc.tile_pool(name="ap", bufs=2, space="PSUM"))
        epool = ctx.enter_context(tc.tile_pool(name="ep", bufs=16))
        rpool = ctx.enter_context(tc.tile_pool(name="rp", bufs=2))
        opool = ctx.enter_context(tc.tile_pool(name="op", bufs=3))

        xt_r = xt_d.rearrange("(k p) n -> p k n", p=128)

        # warmup matmuls (see warm_sb above): accumulate garbage into one
        # transient PSUM slot, freed before the first qk projection needs it.
        wp = apool.tile([128, 512], f32, name="wp", tag="av")
        for i in range(8):
            nc.tensor.matmul(wp[:], warm_sb[:, 0:128], warm_sb[:],
                             start=True, stop=True)

        # prefetch every group's x tile up front so no matmul ever waits on
        # an input DMA mid-stream.
        xtiles = {}
        for g in range(NG):
            xtile = xpool.tile([128, 4, GQ], bf16, name="xtile", tag="xtile")
            xtiles[g] = xtile
            for k in range(4):
                nc.sync.dma_start(xtile[:, k, :],
                                  xt_r[:, k, GQ * g:GQ * (g + 1)])

        next_qp = [0]
        next_kp = [0]
        next_v = [0]

        def emit_qp(g):
            xtile = xtiles[g]
            gsl = slice(GQ * g, GQ * (g + 1))
            qkp = apool.tile([128, 512], f32, name="qkp", tag="av")
            for k in range(4):
                nc.tensor.matmul(qkp[:], wqk_sb[:, k, 0:128],
                                 xtile[:, k, :], start=(k == 0), stop=(k == 3))
            nc.vector.tensor_scalar_add(qT[:, gsl], qkp[:], bqk_sb[:, 0:1])
            nc.vector.tensor_copy(qT2[64:128, gsl], qT[0:64, gsl])
            nc.vector.tensor_copy(qT2[0:64, gsl], qT[64:128, gsl])

        def emit_kp(g):
            xtile = xtiles[g]
            gsl = slice(GQ * g, GQ * (g + 1))
            kkp = apool.tile([128, 512], f32, name="kkp", tag="av")
            for k in range(4):
                nc.tensor.matmul(kkp[:], wqk_sb[:, k, 128:256],
                                 xtile[:, k, :], start=(k == 0), stop=(k == 3))
            nc.vector.tensor_scalar_add(kT[:, gsl], kkp[:], bqk_sb[:, 1:2])
            nc.vector.tensor_copy(kT2[64:128, gsl], kT[0:64, gsl])
            nc.vector.tensor_copy(kT2[0:64, gsl], kT[64:128, gsl])

        def emit_v(m):
            g, t = divmod(m, 4)
            xtile = xtiles[g]
            vp = apool.tile([128, 512], f32, name="vp", tag="av")
            for k in range(4):
                nc.tensor.matmul(vp[:, 0:128],
                                 xtile[:, k, 128 * t:128 * (t + 1)],
                                 wv_sb[:, k, :],
                                 start=(k == 0), stop=(k == 3))
            src = vp[:, 0:128].rearrange("p (a b) -> p a b", a=2)
            dst = vsb[:, m, :].rearrange("p (a b) -> p a b", a=2)
            bvv = bv_sb.rearrange("p (a b) -> p a b", a=2)
            nc.vector.tensor_add(dst[:, :, 0:64], src, bvv)

        def need_qp(g):
            while next_qp[0] <= g:
                emit_qp(next_qp[0])
                next_qp[0] += 1

        def need_kp(g):
            while next_kp[0] <= g:
                emit_kp(next_kp[0])
                next_kp[0] += 1

        def need_v(m):
            while next_v[0] <= m:
                emit_v(next_v[0])
                next_v[0] += 1

        def vol_v(n):
            # voluntary v-projection pops: spread the 32 m-tiles' v matmuls
            # across chunks so they never clump into an ACT-starving burst.
            # (v only needs the prefetched x tiles, not the q/k projections.)
            for _ in range(n):
                m = next_v[0]
                if m < MT:
                    emit_v(m)
                    next_v[0] += 1

        chunks = [list(range(c, min(c + CH, MT))) for c in range(0, MT, CH)]
        items = [(g, h, ms) for g in range(NG) for h in (0, 1) for ms in chunks]

        av_tiles = {}
        proj_boxes = {}
        pending = []
        tick = [0]

        def emit_scores(g, h, ms):
            need_qp(g)
            need_kp(max(ms[-1] // 4, g))
            st = spool.tile([128, 3 * GQ], f32, name="st", tag="sch")
            for j, m in enumerate(ms):
                # alternate PE row groups per m-tile: even m contracts in
                # rows 64h:64h+64 of qT/kT, odd m in the complementary rows
                # of the head-swapped copies -> adjacent matmuls occupy
                # disjoint row groups and run concurrently.
                if m % 2 == 0:
                    kk, qq, r0 = kT, qT, 64 * h
                else:
                    kk, qq, r0 = kT2, qT2, 64 * (1 - h)
                nc.tensor.matmul(st[:, 512 * j:512 * (j + 1)],
                                 kk[r0:r0 + 64, 128 * m:128 * (m + 1)],
                                 qq[r0:r0 + 64, GQ * g:GQ * (g + 1)],
                                 start=True, stop=True)
            et = epool.tile([128, 3 * GQ], bf16, name="et", tag="et")
            w = 512 * len(ms)
            nc.scalar.activation(et[:, 0:w], st[:, 0:w], EXP, scale=SCALE)
            # pop a deferred proj piece every other chunk: each piece is a
            # PE->DVE->PE round-trip through one PSUM slot (~2us), so 1/chunk
            # (1.57us) stalls the strict-order PE queue at g boundaries.
            tick[0] += 1
            if pending and tick[0] % 2 == 0:
                pending.pop(0)()
            if debug and g == 0 and h == 0 and ms[0] == 0:
                nc.sync.dma_start(dbg_e[:], et[:])
            return et

        def emit_post(g, h):
            a = av_tiles.pop((g, h))
            # drain AV psum to SBUF right away: frees the psum slot for the
            # next (g, h) accumulation without waiting on the reciprocal.
            asb = rpool.tile([65, 512], bf16, name="asb", tag="asb")
            nc.vector.tensor_copy(asb[:], a[0:65, :])
            if debug and g == 0 and h == 0:
                nc.sync.dma_start(dbg_at[:], asb[0:64, :])
            box = proj_boxes.setdefault(g, {})
            rbox = {}

            def rtp_piece():
                # transpose denominator row [1, 512] -> [128, 4] via N=1
                # matmuls so the reciprocal runs on 128 lanes instead of 1.
                rtp = apool.tile([128, 512], f32, name="rtp", tag="av")
                for t in range(4):
                    nc.tensor.matmul(rtp[:, t:t + 1],
                                     asb[64:65, 128 * t:128 * (t + 1)],
                                     ones_sb[64:65, 0:1], start=True, stop=True)
                rts = rpool.tile([128, 4], f32, name="rts", tag="rts")
                nc.vector.reciprocal(rts[:], rtp[:, 0:4])
                rbox["r"] = rts

            if g == NG - 1 and h == 1:
                while pending:
                    pending.pop(0)()
                rtp_piece()
                for t in range(4):
                    proj_h1(g, asb, rbox["r"], t, box, act=True)
            else:
                pending.append(rtp_piece)
                fn = proj_h0 if h == 0 else proj_h1
                for t in range(4):
                    pending.append(
                        lambda t=t, fn=fn: fn(g, asb, rbox["r"], t, box))

        def proj_h0(g, a0, r0, t, box, act=False):
            pp0 = apool.tile([128, 512], f32, name="pp0", tag="av")
            nc.tensor.matmul(pp0[:], a0[0:64, 128 * t:128 * (t + 1)],
                             pw_sb[0:64, 0:512], start=True, stop=True)
            t0 = opool.tile([128, 512], bf16, name="t0", tag="t0")
            if act:
                nc.scalar.mul(t0[:], pp0[:], r0[:, t:t + 1])
            else:
                nc.vector.tensor_scalar_mul(t0[:], pp0[:], r0[:, t:t + 1])
            box[t] = t0

        def proj_h1(g, a1, r1, t, box, act=False):
            pp1 = apool.tile([128, 512], f32, name="pp1", tag="av")
            nc.tensor.matmul(pp1[:], a1[0:64, 128 * t:128 * (t + 1)],
                             pw_sb[0:64, 512:1024], start=True, stop=True)
            po = opool.tile([128, 512], f32, name="po", tag="po")
            if act:
                t1 = opool.tile([128, 512], bf16, name="t1", tag="t1")
                nc.scalar.mul(t1[:], pp1[:], r1[:, t:t + 1])
                nc.vector.tensor_add(po[:], box.pop(t), t1[:])
            else:
                # fused (pp1 * r1) + t0 in one DVE op: halves the DVE work
                # per piece and shortens the PSUM round-trip that gates the
                # strict-order PE queue.
                nc.vector.scalar_tensor_tensor(
                    po[:], pp1[:], r1[:, t:t + 1], box.pop(t),
                    mybir.AluOpType.mult, mybir.AluOpType.add)
            nc.sync.dma_start(
                out_d[GQ * g + 128 * t:GQ * g + 128 * (t + 1), :], po[:])

        def emit_av(g, h, ms, et):
            if (g, h) not in av_tiles:
                av_tiles[(g, h)] = apool.tile([128, 512], f32, name="avt",
                                              tag="av")
            a = av_tiles[(g, h)]
            need_v(ms[-1])
            for j, m in enumerate(ms):
                nc.tensor.matmul(a[0:65, :], vsb[:, m, 65 * h:65 * h + 65],
                                 et[:, 512 * j:512 * (j + 1)],
                                 start=(m == 0), stop=(m == MT - 1),
                                 skip_group_check=True)
            if ms[-1] == MT - 1:
                emit_post(g, h)

        from collections import deque
        inflight = deque()
        for ci, it in enumerate(items):
            et = emit_scores(*it)
            inflight.append((it, et))
            # keep the PE dense during item 0 (scores-only chunks are
            # lighter than one exp period; idle accumulates in the HAM
            # window and re-throttles the clock) and prefetch the next
            # group's q-projection mid-item so its DVE copies don't land
            # on the g-boundary critical path.
            if ci >= 11:
                vol_v(3)
            elif ci >= 2:
                vol_v(1)
            if ci % 22 == 13:
                need_qp(min(ci // 22 + 1, NG - 1))
            # AV lags the exp stream by a window that starts wide (so the
            # qk/v projections own the PE during the first item) and decays
            # to 4 (so the tail after the last exp stays short).
            if ci < 24:
                target = 14
            elif ci < 34:
                target = 14 - (ci - 24)
            elif ci < 168:
                # drop the lag to 3 just before each item's last AV chunk:
                # the (g,h0) accumulator then drains two chunks before
                # AV(g,h1) needs its PSUM slot, hiding the DVE CAST latency.
                target = 3 if ci % 22 in (12, 13) else 4
            else:
                target = 2
            while len(inflight) > target:
                (pg, ph, pms), pet = inflight.popleft()
                emit_av(pg, ph, pms, pet)
        while inflight:
            (pg, ph, pms), pet = inflight.popleft()
            emit_av(pg, ph, pms, pet)
        while pending:
            pending.pop(0)()

        if debug:
            nc.sync.dma_start(dbg_qT[:], qT[:])
            nc.sync.dma_start(dbg_kT[:], kT[:])
            nc.sync.dma_start(dbg_v[:], vsb[:])

    nc.compile()
    return nc


def _get_nc():
    if "nc" not in _state:
        _state["nc"] = _build_nc()
    return _state["nc"]


def _make_in_maps(x, qkv_w, qkv_b, proj_w):
    import ml_dtypes
    bf = ml_dtypes.bfloat16
    x = np.asarray(x, np.float32)
    qkv_w = np.asarray(qkv_w, np.float32)
    qkv_b = np.asarray(qkv_b, np.float32)
    proj_w = np.asarray(proj_w, np.float32)
    in_maps = []
    for core in range(8):
        b, hp = divmod(core, 4)
        h0, h1 = 2 * hp, 2 * hp + 1
        xt = np.ascontiguousarray(x[b].T).astype(bf)
        rq = np.concatenate([qkv_w[64 * h0:64 * h0 + 64],
                             qkv_w[64 * h1:64 * h1 + 64]], 0)
        rk = np.concatenate([qkv_w[C + 64 * h0:C + 64 * h0 + 64],
                             qkv_w[C + 64 * h1:C + 64 * h1 + 64]], 0)
        wqk = np.ascontiguousarray(np.concatenate([rq, rk], 0).T).astype(bf)
        bq = np.concatenate([qkv_b[64 * h0:64 * h0 + 64],
                             qkv_b[64 * h1:64 * h1 + 64]])
        bk = np.concatenate([qkv_b[C + 64 * h0:C + 64 * h0 + 64],
                             qkv_b[C + 64 * h1:C + 64 * h1 + 64]])
        bqk = np.ascontiguousarray(np.stack([bq, bk], 1)).astype(np.float32)
        rv = np.concatenate([qkv_w[2 * C + 64 * h0:2 * C + 64 * h0 + 64],
                             qkv_w[2 * C + 64 * h1:2 * C + 64 * h1 + 64]], 0)
        wv = np.ascontiguousarray(rv.T).astype(bf)
        bvrow = np.concatenate([qkv_b[2 * C + 64 * h0:2 * C + 64 * h0 + 64],
                                qkv_b[2 * C + 64 * h1:2 * C + 64 * h1 + 64]])
        bv = np.ascontiguousarray(
            np.broadcast_to(bvrow[None, :], (128, 128))).astype(bf)
        pwT = np.ascontiguousarray(proj_w[:, 128 * hp:128 * hp + 128].T)
        pw2 = np.ascontiguousarray(
            np.concatenate([pwT[0:64], pwT[64:128]], 1)).astype(bf)
        in_maps.append(dict(xt=xt, wqk=wqk, bqk=bqk, wv=wv, bv=bv, pw2=pw2))
    return in_maps


def _gather(results, proj_b):
    proj_b = np.asarray(proj_b, np.float32)
    out = np.empty((B, N, C), np.float32)
    for b in range(B):
        acc = results[4 * b]["out"].astype(np.float32).copy()
        for hp in range(1, 4):
            acc += results[4 * b + hp]["out"]
        out[b] = acc + proj_b[None, :]
    return out


def _run(x, qkv_w, qkv_b, proj_w, proj_b, trace=False, tmpdir=None):
    from concourse import bass_utils
    nc = _get_nc()
    in_maps = _make_in_maps(x, qkv_w, qkv_b, proj_w)
    res = bass_utils.run_bass_kernel_spmd(
        nc, in_maps, core_ids=list(range(8)), trace=trace, tmpdir=tmpdir)
    return _gather(res.results, proj_b), res


def kernel(x, qkv_w, qkv_b, proj_w, proj_b):
    out, _ = _run(x, qkv_w, qkv_b, proj_w, proj_b, trace=False)
    return out



# revision 46
# speedup vs baseline: 1.0021x; 1.0021x over previous
"""Multi-head attention (B=2, N=4096, C=512, H=8) on 8 trn2 NeuronCores.

Sharding: core -> (batch b = core//4, head-pair hp = core%4), i.e. data
parallel over B and tensor parallel over the 8 heads (2 heads per core),
with column-sharded qkv weights and row-sharded proj weights. Each core
returns a partial projection output [4096, 512]; the host sums the 4
head-pair partials per batch and adds proj_b.

Per-core device kernel (flash-style, nothing N^2 ever hits HBM):
  qT/kT  [128(=2 heads x 64 feat), 4096]  <- wqk^T @ x^T   (bf16 matmuls)
  qT2/kT2: head-swapped copies (h0 in partitions 64:128) so consecutive
    key m-tiles contract in disjoint 64-row PE groups -> adjacent K=64
    score matmuls run CONCURRENTLY in the array (row tiling, ~2x scores)
  v_sb   [128 keys, 32 m-tiles, 65]       <- x^T^T @ wv (+bias), ones col
  per (query-group g of 512 queries, head h), chunks of CH=3 key m-tiles:
    S^T chunk [128 keys, 3*512 q] in PSUM <- kT_m-x-qT  (scores matmuls)
    E = exp(SCALE * S^T) on the ACT engine -> SBUF bf16 (one ACTIVATE/chunk;
    the ACT engine is the throughput wall: 33.5M exps/core at 1 elem/
    cycle/lane x 1.2 GHz ~= 218us + 293ns/ACTIVATE overhead)
    out^T [65, 512] PSUM += v_aug-x-E  (row 64 = softmax denominator, free
    via the ones column; accumulation lags the exp stream by a skew that
    starts at 14 chunks -- so the qk/v projections own the PE during item
    0 -- and decays to 2)
  per (g, h) tail, deferred via a pending queue popped every OTHER chunk
  (each piece is a PE->DVE->PE round-trip through one PSUM slot; popping
  faster stalls the strict-order PE queue and HAM-cools the clock):
    drain out^T to SBUF; transpose denom row to [128, 4] with N=1 matmuls;
    128-lane reciprocal; per-head proj of the UNNORMALIZED out^T; fused
    scale-by-1/denom + head-sum via scalar_tensor_tensor on DVE; DMA out.

Scheduling notes (measured on HW): 8 warmup matmuls release the HAM
clock gate during the ~8.5us queue-boot window; all x tiles prefetch up
front; q/k/v projections are demand-paced (kproj by the key-m frontier,
vproj voluntarily at 1/chunk from chunk 2 -- item-0 chunks must stay PE-
dense or idle accumulates in the HAM window and re-throttles the clock --
then 3/chunk from chunk 11). The next group's q-projection prefetches at
ci%22==13 so its DVE copies clear before the group boundary.
PSUM budget: 6 banks score double-buffer + 2 transient/AV banks = 8.
The device power-throttles run-to-run (ACT busy 257 vs 308us bimodal);
compare only runs with similar scalar-engine busy time.
"""

import numpy as np

_state = {}

B, N, C, H, DH = 2, 4096, 512, 8, 64
SCALE = DH ** -0.5
GQ = 512          # queries per group
NG = N // GQ      # 8 groups
MT = N // 128     # 32 key m-tiles
CH = 3            # m-tiles per exp chunk


def _build_nc(debug=False):
    from contextlib import ExitStack

    import concourse.bacc as bacc
    import concourse.tile as tile
    from concourse import mybir

    bf16 = mybir.dt.bfloat16
    f32 = mybir.dt.float32
    f32r = mybir.dt.float32r
    EXP = mybir.ActivationFunctionType.Exp

    nc = bacc.Bacc(None, target_bir_lowering=False)
    with tile.TileContext(nc) as tc, ExitStack() as ctx:
        dram = ctx.enter_context(tc.tile_pool(name="dram", bufs=1, space="DRAM"))
        xt_d = dram.tile([C, N], bf16, kind="ExternalInput", name="xt",
                         uniquify=False, tag="dxt")
        wqk_d = dram.tile([C, 256], bf16, kind="ExternalInput", name="wqk",
                          uniquify=False, tag="dwqk")
        bqk_d = dram.tile([128, 2], f32, kind="ExternalInput", name="bqk",
                          uniquify=False, tag="dbqk")
        wv_d = dram.tile([C, 128], bf16, kind="ExternalInput", name="wv",
                         uniquify=False, tag="dwv")
        bv_d = dram.tile([128, 128], bf16, kind="ExternalInput", name="bv",
                         uniquify=False, tag="dbv")
        pw_d = dram.tile([64, 1024], bf16, kind="ExternalInput", name="pw2",
                         uniquify=False, tag="dpw")
        out_d = dram.tile([N, C], f32, kind="ExternalOutput", name="out",
                          uniquify=False, tag="dout")
        if debug:
            dbg_qT = dram.tile([128, N], bf16, kind="ExternalOutput",
                               name="dbg_qT", uniquify=False, tag="dbg_qT")
            dbg_kT = dram.tile([128, N], bf16, kind="ExternalOutput",
                               name="dbg_kT", uniquify=False, tag="dbg_kT")
            dbg_v = dram.tile([128, MT, 130], bf16, kind="ExternalOutput",
                              name="dbg_v", uniquify=False, tag="dbg_v")
            dbg_e = dram.tile([128, 3 * GQ], bf16, kind="ExternalOutput",
                              name="dbg_e", uniquify=False, tag="dbg_e")
            dbg_at = dram.tile([64, GQ], bf16, kind="ExternalOutput",
                               name="dbg_at", uniquify=False, tag="dbg_at")
            dbg_rb = dram.tile([1, GQ], bf16, kind="ExternalOutput",
                               name="dbg_rb", uniquify=False, tag="dbg_rb")

        const = ctx.enter_context(tc.tile_pool(name="const", bufs=1))
        wqk_sb = const.tile([128, 4, 256], bf16, name="wqk_sb", tag="wqk_sb")
        nc.gpsimd.dma_start(wqk_sb[:], wqk_d.rearrange("(k p) f -> p k f", p=128))
        wv_sb = const.tile([128, 4, 128], bf16, name="wv_sb", tag="wv_sb")
        nc.gpsimd.dma_start(wv_sb[:], wv_d.rearrange("(k p) f -> p k f", p=128))
        bqk_sb = const.tile([128, 2], f32, name="bqk_sb", tag="bqk_sb")
        nc.gpsimd.dma_start(bqk_sb[:], bqk_d[:])
        bv_sb = const.tile([128, 128], bf16, name="bv_sb", tag="bv_sb")
        nc.gpsimd.dma_start(bv_sb[:], bv_d[:])
        pw_sb = const.tile([64, 1024], bf16, name="pw_sb", tag="pw_sb")
        nc.gpsimd.dma_start(pw_sb[:], pw_d[:])
        ones_sb = const.tile([65, 128], bf16, name="ones_sb", tag="ones_sb")
        nc.vector.memset(ones_sb[:], 1.0)
        # PE warmup: ~34 dummy matmuls on a self-contained SBUF tile keep the
        # PE busy through the ~9us DMA/boot window so the HAM clock gate is
        # released (2.4 GHz) before the first real matmul issues.
        warm_sb = const.tile([128, 512], bf16, name="warm_sb", tag="warm_sb")
        nc.vector.memset(warm_sb[:], 1.0)

        persist = ctx.enter_context(tc.tile_pool(name="persist", bufs=1))
        qT = persist.tile([128, N], bf16, name="qT", tag="qT")
        kT = persist.tile([128, N], bf16, name="kT", tag="kT")
        # head-swapped copies (h0 in partitions 64:128, h1 in 0:64): lets
        # consecutive key m-tiles use disjoint 64-row PE groups so their
        # K=64 score matmuls run concurrently in the array (row tiling).
        qT2 = persist.tile([128, N], bf16, name="qT2", tag="qT2")
        kT2 = persist.tile([128, N], bf16, name="kT2", tag="kT2")
        vsb = persist.tile([128, MT, 130], bf16, name="vsb", tag="vsb")
        vones = vsb.rearrange("p m (a b) -> p m a b", a=2)
        nc.vector.memset(vones[:, :, 0, 64:65], 1.0)
        nc.vector.memset(vones[:, :, 1, 64:65], 1.0)

        xpool = ctx.enter_context(tc.tile_pool(name="xp", bufs=8))
        spool = ctx.enter_context(tc.tile_pool(name="sp", bufs=2, space="PSUM"))
        apool = ctx.enter_context(tc.tile_pool(name="ap", bufs=2, space="PSUM"))
        epool = ctx.enter_context(tc.tile_pool(name="ep", bufs=16))
        rpool = ctx.enter_context(tc.tile_pool(name="rp", bufs=2))
        opool = ctx.enter_context(tc.tile_pool(name="op", bufs=3))

        xt_r = xt_d.rearrange("(k p) n -> p k n", p=128)

        # warmup matmuls (see warm_sb above): accumulate garbage into one
        # transient PSUM slot, freed before the first qk projection needs it.
        wp = apool.tile([128, 512], f32, name="wp", tag="av")
        for i in range(8):
            nc.tensor.matmul(wp[:], warm_sb[:, 0:128], warm_sb[:],
                             start=True, stop=True)

        # prefetch every group's x tile up front so no matmul ever waits on
        # an input DMA mid-stream.
        xtiles = {}
        for g in range(NG):
            xtile = xpool.tile([128, 4, GQ], bf16, name="xtile", tag="xtile")
            xtiles[g] = xtile
            for k in range(4):
                nc.sync.dma_start(xtile[:, k, :],
                                  xt_r[:, k, GQ * g:GQ * (g + 1)])

        next_qp = [0]
        next_kp = [0]
        next_v = [0]

        def emit_qp(g):
            xtile = xtiles[g]
            gsl = slice(GQ * g, GQ * (g + 1))
            qkp = apool.tile([128, 512], f32, name="qkp", tag="av")
            for k in range(4):
                nc.tensor.matmul(qkp[:], wqk_sb[:, k, 0:128],
                                 xtile[:, k, :], start=(k == 0), stop=(k == 3))
            nc.vector.tensor_scalar_add(qT[:, gsl], qkp[:], bqk_sb[:, 0:1])
            nc.vector.tensor_copy(qT2[64:128, gsl], qT[0:64, gsl])
            nc.vector.tensor_copy(qT2[0:64, gsl], qT[64:128, gsl])

        def emit_kp(g):
            xtile = xtiles[g]
            gsl = slice(GQ * g, GQ * (g + 1))
            kkp = apool.tile([128, 512], f32, name="kkp", tag="av")
            for k in range(4):
                nc.tensor.matmul(kkp[:], wqk_sb[:, k, 128:256],
                                 xtile[:, k, :], start=(k == 0), stop=(k == 3))
            nc.vector.tensor_scalar_add(kT[:, gsl], kkp[:], bqk_sb[:, 1:2])
            nc.vector.tensor_copy(kT2[64:128, gsl], kT[0:64, gsl])
            nc.vector.tensor_copy(kT2[0:64, gsl], kT[64:128, gsl])

        def emit_v(m):
            g, t = divmod(m, 4)
            xtile = xtiles[g]
            vp = apool.tile([128, 512], f32, name="vp", tag="av")
            for k in range(4):
                nc.tensor.matmul(vp[:, 0:128],
                                 xtile[:, k, 128 * t:128 * (t + 1)],
                                 wv_sb[:, k, :],
                                 start=(k == 0), stop=(k == 3))
            src = vp[:, 0:128].rearrange("p (a b) -> p a b", a=2)
            dst = vsb[:, m, :].rearrange("p (a b) -> p a b", a=2)
            bvv = bv_sb.rearrange("p (a b) -> p a b", a=2)
            nc.vector.tensor_add(dst[:, :, 0:64], src, bvv)

        def need_qp(g):
            while next_qp[0] <= g:
                emit_qp(next_qp[0])
                next_qp[0] += 1

        def need_kp(g):
            while next_kp[0] <= g:
                emit_kp(next_kp[0])
                next_kp[0] += 1

        def need_v(m):
            while next_v[0] <= m:
                emit_v(next_v[0])
                next_v[0] += 1

        def vol_v(n):
            # voluntary v-projection pops: spread the 32 m-tiles' v matmuls
            # across chunks so they never clump into an ACT-starving burst.
            # (v only needs the prefetched x tiles, not the q/k projections.)
            for _ in range(n):
                m = next_v[0]
                if m < MT:
                    emit_v(m)
                    next_v[0] += 1

        chunks = [list(range(c, min(c + CH, MT))) for c in range(0, MT, CH)]
        items = [(g, h, ms) for g in range(NG) for h in (0, 1) for ms in chunks]

        av_tiles = {}
        proj_boxes = {}
        pending = []
        tick = [0]

        def emit_scores(g, h, ms):
            need_qp(g)
            need_kp(max(ms[-1] // 4, g))
            st = spool.tile([128, 3 * GQ], f32, name="st", tag="sch")
            for j, m in enumerate(ms):
                # alternate PE row groups per m-tile: even m contracts in
                # rows 64h:64h+64 of qT/kT, odd m in the complementary rows
                # of the head-swapped copies -> adjacent matmuls occupy
                # disjoint row groups and run concurrently.
                if m % 2 == 0:
                    kk, qq, r0 = kT, qT, 64 * h
                else:
                    kk, qq, r0 = kT2, qT2, 64 * (1 - h)
                nc.tensor.matmul(st[:, 512 * j:512 * (j + 1)],
                                 kk[r0:r0 + 64, 128 * m:128 * (m + 1)],
                                 qq[r0:r0 + 64, GQ * g:GQ * (g + 1)],
                                 start=True, stop=True)
            et = epool.tile([128, 3 * GQ], bf16, name="et", tag="et")
            w = 512 * len(ms)
            nc.scalar.activation(et[:, 0:w], st[:, 0:w], EXP, scale=SCALE)
            # pop a deferred proj piece every other chunk: each piece is a
            # PE->DVE->PE round-trip through one PSUM slot (~2us), so 1/chunk
            # (1.57us) stalls the strict-order PE queue at g boundaries.
            tick[0] += 1
            if pending and tick[0] % 2 == 0:
                pending.pop(0)()
            if debug and g == 0 and h == 0 and ms[0] == 0:
                nc.sync.dma_start(dbg_e[:], et[:])
            return et

        def emit_post(g, h):
            a = av_tiles.pop((g, h))
            # drain AV psum to SBUF right away: frees the psum slot for the
            # next (g, h) accumulation without waiting on the reciprocal.
            asb = rpool.tile([65, 512], bf16, name="asb", tag="asb")
            nc.vector.tensor_copy(asb[:], a[0:65, :])
            if debug and g == 0 and h == 0:
                nc.sync.dma_start(dbg_at[:], asb[0:64, :])
            box = proj_boxes.setdefault(g, {})
            rbox = {}

            def rtp_piece():
                # transpose denominator row [1, 512] -> [128, 4] via N=1
                # matmuls so the reciprocal runs on 128 lanes instead of 1.
                rtp = apool.tile([128, 512], f32, name="rtp", tag="av")
                for t in range(4):
                    nc.tensor.matmul(rtp[:, t:t + 1],
                                     asb[64:65, 128 * t:128 * (t + 1)],
                                     ones_sb[64:65, 0:1], start=True, stop=True)
                rts = rpool.tile([128, 4], f32, name="rts", tag="rts")
                nc.vector.reciprocal(rts[:], rtp[:, 0:4])
                rbox["r"] = rts

            if g == NG - 1 and h == 1:
                while pending:
                    pending.pop(0)()
                rtp_piece()
                for t in range(4):
                    proj_h1(g, asb, rbox["r"], t, box, act=True)
            else:
                pending.append(rtp_piece)
                fn = proj_h0 if h == 0 else proj_h1
                for t in range(4):
                    pending.append(
                        lambda t=t, fn=fn: fn(g, asb, rbox["r"], t, box))

        def proj_h0(g, a0, r0, t, box, act=False):
            pp0 = apool.tile([128, 512], f32, name="pp0", tag="av")
            nc.tensor.matmul(pp0[:], a0[0:64, 128 * t:128 * (t + 1)],
                             pw_sb[0:64, 0:512], start=True, stop=True)
            t0 = opool.tile([128, 512], bf16, name="t0", tag="t0")
            if act:
                nc.scalar.mul(t0[:], pp0[:], r0[:, t:t + 1])
            else:
                nc.vector.tensor_scalar_mul(t0[:], pp0[:], r0[:, t:t + 1])
            box[t] = t0

        def proj_h1(g, a1, r1, t, box, act=False):
            pp1 = apool.tile([128, 512], f32, name="pp1", tag="av")
            nc.tensor.matmul(pp1[:], a1[0:64, 128 * t:128 * (t + 1)],
                             pw_sb[0:64, 512:1024], start=True, stop=True)
            po = opool.tile([128, 512], f32, name="po", tag="po")
            if act:
                t1 = opool.tile([128, 512], bf16, name="t1", tag="t1")
                nc.scalar.mul(t1[:], pp1[:], r1[:, t:t + 1])
                nc.vector.tensor_add(po[:], box.pop(t), t1[:])
            else:
                # fused (pp1 * r1) + t0 in one DVE op: halves the DVE work
                # per piece and shortens the PSUM round-trip that gates the
                # strict-order PE queue.
                nc.vector.scalar_tensor_tensor(
                    po[:], pp1[:], r1[:, t:t + 1], box.pop(t),
                    mybir.AluOpType.mult, mybir.AluOpType.add)
            nc.sync.dma_start(
                out_d[GQ * g + 128 * t:GQ * g + 128 * (t + 1), :], po[:])

        def emit_av(g, h, ms, et):
            if (g, h) not in av_tiles:
                av_tiles[(g, h)] = apool.tile([128, 512], f32, name="avt",
                                              tag="av")
            a = av_tiles[(g, h)]
            need_v(ms[-1])
            for j, m in enumerate(ms):
                nc.tensor.matmul(a[0:65, :], vsb[:, m, 65 * h:65 * h + 65],
                                 et[:, 512 * j:512 * (j + 1)],
                                 start=(m == 0), stop=(m == MT - 1),
                                 skip_group_check=True)
            if ms[-1] == MT - 1:
                emit_post(g, h)

        from collections import deque
        inflight = deque()
        for ci, it in enumerate(items):
            et = emit_scores(*it)
            inflight.append((it, et))
            # keep the PE dense during item 0 (scores-only chunks are
            # lighter than one exp period; idle accumulates in the HAM
            # window and re-throttles the clock) and prefetch the next
            # group's q-projection mid-item so its DVE copies don't land
            # on the g-boundary critical path.
            if ci >= 11:
                vol_v(3)
            elif ci >= 2:
                vol_v(1)
            if ci % 22 == 13:
                need_qp(min(ci // 22 + 1, NG - 1))
            # AV lags the exp stream by a window that starts wide (so the
            # qk/v projections own the PE during the first item) and decays
            # to 4 (so the tail after the last exp stays short).
            if ci < 24:
                target = 14
            elif ci < 34:
                target = 14 - (ci - 24)
            elif ci < 168:
                # drop the lag to 3 just before each item's last AV chunk:
                # the (g,h0) accumulator then drains two chunks before
                # AV(g,h1) needs its PSUM slot, hiding the DVE CAST latency.
                target = 3 if ci % 22 in (12, 13) else 4
            else:
                target = 2
            while len(inflight) > target:
                (pg, ph, pms), pet = inflight.popleft()
                emit_av(pg, ph, pms, pet)
        while inflight:
            (pg, ph, pms), pet = inflight.popleft()
            emit_av(pg, ph, pms, pet)
        while pending:
            pending.pop(0)()

        if debug:
            nc.sync.dma_start(dbg_qT[:], qT[:])
            nc.sync.dma_start(dbg_kT[:], kT[:])
            nc.sync.dma_start(dbg_v[:], vsb[:])

    nc.compile()
    return nc


def _get_nc():
    if "nc" not in _state:
        _state["nc"] = _build_nc()
    return _state["nc"]


def _make_in_maps(x, qkv_w, qkv_b, proj_w):
    import ml_dtypes
    bf = ml_dtypes.bfloat16
    x = np.asarray(x, np.float32)
    qkv_w = np.asarray(qkv_w, np.float32)
    qkv_b = np.asarray(qkv_b, np.float32)
    proj_w = np.asarray(proj_w, np.float32)
    in_maps = []
    for core in range(8):
        b, hp = divmod(core, 4)
        h0, h1 = 2 * hp, 2 * hp + 1
        xt = np.ascontiguousarray(x[b].T).astype(bf)
        rq = np.concatenate([qkv_w[64 * h0:64 * h0 + 64],
                             qkv_w[64 * h1:64 * h1 + 64]], 0)
        rk = np.concatenate([qkv_w[C + 64 * h0:C + 64 * h0 + 64],
                             qkv_w[C + 64 * h1:C + 64 * h1 + 64]], 0)
        wqk = np.ascontiguousarray(np.concatenate([rq, rk], 0).T).astype(bf)
        bq = np.concatenate([qkv_b[64 * h0:64 * h0 + 64],
                             qkv_b[64 * h1:64 * h1 + 64]])
        bk = np.concatenate([qkv_b[C + 64 * h0:C + 64 * h0 + 64],
                             qkv_b[C + 64 * h1:C + 64 * h1 + 64]])
        bqk = np.ascontiguousarray(np.stack([bq, bk], 1)).astype(np.float32)
        rv = np.concatenate([qkv_w[2 * C + 64 * h0:2 * C + 64 * h0 + 64],
                             qkv_w[2 * C + 64 * h1:2 * C + 64 * h1 + 64]], 0)
        wv = np.ascontiguousarray(rv.T).astype(bf)
        bvrow = np.concatenate([qkv_b[2 * C + 64 * h0:2 * C + 64 * h0 + 64],
                                qkv_b[2 * C + 64 * h1:2 * C + 64 * h1 + 64]])
        bv = np.ascontiguousarray(
            np.broadcast_to(bvrow[None, :], (128, 128))).astype(bf)
        pwT = np.ascontiguousarray(proj_w[:, 128 * hp:128 * hp + 128].T)
        pw2 = np.ascontiguousarray(
            np.concatenate([pwT[0:64], pwT[64:128]], 1)).astype(bf)
        in_maps.append(dict(xt=xt, wqk=wqk, bqk=bqk, wv=wv, bv=bv, pw2=pw2))
    return in_maps


def _gather(results, proj_b):
    proj_b = np.asarray(proj_b, np.float32)
    out = np.empty((B, N, C), np.float32)
    for b in range(B):
        acc = results[4 * b]["out"].astype(np.float32).copy()
        for hp in range(1, 4):
            acc += results[4 * b + hp]["out"]
        out[b] = acc + proj_b[None, :]
    return out


def _run(x, qkv_w, qkv_b, proj_w, proj_b, trace=False, tmpdir=None):
    from concourse import bass_utils
    nc = _get_nc()
    in_maps = _make_in_maps(x, qkv_w, qkv_b, proj_w)
    res = bass_utils.run_bass_kernel_spmd(
        nc, in_maps, core_ids=list(range(8)), trace=trace, tmpdir=tmpdir)
    return _gather(res.results, proj_b), res


def kernel(x, qkv_w, qkv_b, proj_w, proj_b):
    out, _ = _run(x, qkv_w, qkv_b, proj_w, proj_b, trace=False)
    return out

